# revision 1
# baseline (speedup 1.0000x reference)
"""F1-score (macro) kernel for Trainium2, 8 NeuronCores.

Per core (data-parallel over rows), tiles of TK*128 rows ([128p, TK, 128c],
row = base + p*TK + k):
  - DVE:  rowmax via tensor_reduce (X axis, split in halves)
  - ACT:  anti-one-hot(pred) = sign(rowmax - x) in {0,1}  (most chunks)
  - GS :  a couple of anti chunks via is_lt
  - DVE:  one-hot(true) = (iota == y_true) via broadcast-TT ranges
  - PE :  cm_dev += one_hot_trueT @ anti  (bf16 matmuls, fp32 PSUM)
Host: cm = support[t] - sum_cores(cm_dev);  macro-F1 epilogue on [128,128].
All comparisons in exact fp32 -> bit-exact confusion matrix.
"""

import sys
import time

if "/opt/trn_rl_repo" not in sys.path:
    sys.path.insert(0, "/opt/trn_rl_repo")

import numpy as np

import concourse.bacc as bacc
import concourse.mybir as mybir
import concourse.tile as tile
from concourse import bass_utils

C = 128
N = 1_000_000
NCORES = 8
R = N // NCORES          # 125000 rows per core
TK = 16                  # chunks (of 128 rows) per big tile
TR = 128 * TK            # 4096 rows per big tile
NT = R // TR             # 30 big tiles
MID = (R - NT * TR) // 128   # leftover full chunks (16)
TAIL = R - NT * TR - MID * 128  # 72 rows
EPS = 1e-12

N_GS = 1                 # trailing anti chunks per big tile on GpSimd

_CACHE = {}


def _build():
    f32 = mybir.dt.float32
    bf16 = mybir.dt.bfloat16
    Alu = mybir.AluOpType
    Act = mybir.ActivationFunctionType

    nc = bacc.Bacc("TRN2", target_bir_lowering=False, debug=False,
                   num_devices=NCORES)
    yp = nc.dram_tensor("yp", [R, C], f32, kind="ExternalInput")
    yt = nc.dram_tensor("yt", [R], f32, kind="ExternalInput")
    cm = nc.dram_tensor("cm", [C, C], f32, kind="ExternalOutput")

    with tile.TileContext(nc) as tc:
        with (
            tc.tile_pool(name="const", bufs=1) as cpool,
            tc.tile_pool(name="xin", bufs=5) as xpool,
            tc.tile_pool(name="oh", bufs=6) as ohpool,
            tc.tile_pool(name="small", bufs=6) as spool,
            tc.tile_pool(name="psum", bufs=1, space="PSUM") as psum,
        ):
            iota_i = cpool.tile([128, C], mybir.dt.int32)
            nc.gpsimd.iota(iota_i[:], pattern=[[1, C]], base=0,
                           channel_multiplier=0)
            iota_bf = cpool.tile([128, C], bf16)
            nc.vector.tensor_copy(iota_bf[:], iota_i[:])
            iota_rep = cpool.tile([128, TK, C], bf16)
            nc.vector.tensor_copy(
                iota_rep[:], iota_bf[:, None, :].broadcast_to([128, TK, C])
            )

            acc = psum.tile([C, C], f32)
            state = {"started": False}

            def emit_tile(base, tk, n_gs):
                """One tile of 128*tk rows at row `base`: row = base+p*tk+k."""
                x = xpool.tile([128, tk, C], f32, tag="x")
                nc.sync.dma_start(
                    x[:],
                    yp.ap()[base : base + 128 * tk, :].rearrange(
                        "(p k) c -> p k c", k=tk
                    ),
                )
                t = spool.tile([128, tk], f32, tag="t")
                nc.sync.dma_start(
                    t[:],
                    yt.ap()[base : base + 128 * tk].rearrange(
                        "(p k) -> p k", k=tk
                    ),
                )
                rmax = spool.tile([128, tk], f32, tag="rmax")
                anti = ohpool.tile([128, tk, C], bf16, tag="anti")
                oht = ohpool.tile([128, tk, C], bf16, tag="oht")
                # oht first: depends only on the small y_true DMA, so DVE can
                # build it while the 1MB x DMA is still streaming in.
                nc.vector.tensor_tensor(
                    oht[:, 0:tk, :], iota_rep[:, 0:tk, :],
                    t[:, 0:tk, None].broadcast_to([128, tk, C]),
                    op=Alu.is_equal,
                )
                nc.vector.tensor_reduce(
                    rmax[:], x[:], axis=mybir.AxisListType.X, op=Alu.max
                )
                for k in range(tk - n_gs):
                    nc.scalar.activation(
                        anti[:, k, :], x[:, k, :], Act.Sign,
                        bias=rmax[:, k : k + 1], scale=-1.0,
                    )
                for k in range(tk - n_gs, tk):
                    nc.gpsimd.tensor_scalar(
                        anti[:, k, :], x[:, k, :], rmax[:, k : k + 1], None,
                        op0=Alu.is_lt,
                    )
                for k in range(tk):
                    nc.tensor.matmul(
                        acc[:], oht[:, k, :], anti[:, k, :],
                        start=not state["started"], stop=False,
                    )
                    state["started"] = True

            for i in range(NT):
                emit_tile(i * TR, TK, N_GS)
            if MID:
                emit_tile(NT * TR, MID, 1)

            # tail rows (72), all on DVE
            base = NT * TR + MID * 128
            xt = xpool.tile([TAIL, 1, C], f32, tag="xtail")
            nc.sync.dma_start(
                xt[:],
                yp.ap()[base : R, :].rearrange("(p k) c -> p k c", k=1),
            )
            tt = spool.tile([TAIL, 1], f32, tag="ttail")
            nc.sync.dma_start(
                tt[:], yt.ap()[base : R].rearrange("(p k) -> p k", k=1)
            )
            rmax_t = spool.tile([TAIL, 1], f32, tag="rmaxtail")
            nc.vector.tensor_reduce(
                rmax_t[:], xt[:], axis=mybir.AxisListType.X, op=Alu.max
            )
            anti_t = ohpool.tile([TAIL, C], bf16, tag="antitail")
            oht_t = ohpool.tile([TAIL, C], bf16, tag="ohttail")
            nc.vector.tensor_scalar(
                anti_t[:], xt[:, 0, :], rmax_t[:], None, op0=Alu.is_lt
            )
            nc.vector.tensor_scalar(
                oht_t[:], iota_bf[:TAIL, :], tt[:], None, op0=Alu.is_equal
            )
            nc.tensor.matmul(
                acc[:], oht_t[:], anti_t[:], start=False, stop=True
            )

            out_sb = spool.tile([C, C], f32, tag="out")
            nc.scalar.copy(out_sb[:], acc[:])
            nc.sync.dma_start(cm.ap()[:], out_sb[:])

    nc.compile()
    return nc


def _get_nc():
    if "nc" not in _CACHE:
        _CACHE["nc"] = _build()
    return _CACHE["nc"]


def _run(y_pred, y_true, trace=False):
    nc = _get_nc()
    y_pred = np.ascontiguousarray(np.asarray(y_pred, dtype=np.float32))
    yt_i = np.asarray(y_true).astype(np.int64)
    yt_f = yt_i.astype(np.float32)
    in_maps = [
        {
            "yp": y_pred[c * R : (c + 1) * R],
            "yt": np.ascontiguousarray(yt_f[c * R : (c + 1) * R]),
        }
        for c in range(NCORES)
    ]
    res = None
    for attempt in range(3):
        try:
            res = bass_utils.run_bass_kernel_spmd(
                nc, in_maps, core_ids=list(range(NCORES)), trace=trace
            )
            break
        except Exception:
            if attempt == 2:
                raise
            time.sleep(2.0)
    cm_dev = np.zeros((C, C), dtype=np.float64)
    for r in res.results:
        cm_dev += r["cm"].astype(np.float64)
    support = np.bincount(yt_i, minlength=C).astype(np.float64)
    cm = support[:, None] - cm_dev
    diag = np.diagonal(cm)
    precision = diag / (cm.sum(axis=1) + EPS)
    recall = diag / (cm.sum(axis=0) + EPS)
    f1 = 2.0 * precision * recall / (precision + recall + EPS)
    return np.float32(f1.mean()), res


def kernel(y_pred, y_true):
    out, _ = _run(y_pred, y_true, trace=False)
    return out



# revision 3
# speedup vs baseline: 1.3627x; 1.3627x over previous
"""Macro-F1 kernel for Trainium2, 8 NeuronCores.

Host-side counting sort groups rows by true class into class-pure 128-row
chunks (each class padded to a multiple of 128 with sentinel rows whose
argmax is exactly class 0).  The device then never needs y_true or a
one-hot build:

Per core, tiles of TK*128 rows laid out [128p, TK, 128c] (physical shard
row = 128*b + p*TK + k holds row p of logical chunk b+k):
  - DVE:  rowmax via tensor_reduce (X axis, one instr per tile)
  - anti[r,p] = (x[r,p] < rowmax[r]) in {0,1}, exact fp32 compare, split
    across engines per chunk: J_DVE chunks as one broadcast tensor_tensor
    on DVE, 1-2 chunks on GpSimd, the rest on ACT (Sign(rowmax - x)).
  - PE :  per chunk one matmul, stationary=anti [128,128] bf16,
          moving=ones [128,1] -> column sums into PSUM slot [:, G] of a
          [128,512] bank.  985 chunks fit in 2 PSUM banks; no mid-kernel
          eviction.
Host: counts[chunk,p] = 128 - colsum_anti; regroup chunk count vectors by
class, subtract the sentinel contributions, fp64 macro-F1 epilogue.
"""

import sys
import time

if "/opt/trn_rl_repo" not in sys.path:
    sys.path.insert(0, "/opt/trn_rl_repo")

import numpy as np

import concourse.bacc as bacc
import concourse.mybir as mybir
import concourse.tile as tile
from concourse import bass_utils

C = 128
NCORES = 8
TK = 32                  # chunks (of 128 rows) per tile
J_DVE = 11               # leading chunks per tile compared on DVE
BIG = np.float32(1e30)   # sentinel rows: [BIG, 0, ..., 0] -> argmax == 0
EPS = 1e-12

_CACHE = {}


def _tiles(M):
    out = []
    b = 0
    while b < M:
        tk = min(TK, M - b)
        out.append((b, tk))
        b += tk
    return out


def _n_gs(ti, tk, j):
    """GpSimd chunks for tile index ti (avg 1.5/tile)."""
    if tk <= j:
        return 0
    return min(2 if ti % 2 == 0 else 1, tk - j)


def _build(M):
    f32 = mybir.dt.float32
    bf16 = mybir.dt.bfloat16
    Alu = mybir.AluOpType
    Act = mybir.ActivationFunctionType

    R = M * 128
    NB = -(-M // 512)    # psum banks used

    nc = bacc.Bacc("TRN2", target_bir_lowering=False, debug=False,
                   num_devices=NCORES)
    yp = nc.dram_tensor("yp", [R, C], f32, kind="ExternalInput")
    out = nc.dram_tensor("out", [NB, C, 512], f32, kind="ExternalOutput")

    with tile.TileContext(nc) as tc:
        with (
            tc.tile_pool(name="const", bufs=1) as cpool,
            tc.tile_pool(name="xin", bufs=6) as xpool,
            tc.tile_pool(name="anti", bufs=3) as apool,
            tc.tile_pool(name="small", bufs=6) as spool,
            tc.tile_pool(name="psum", bufs=1, space="PSUM") as psum,
        ):
            ones = cpool.tile([128, 1], bf16)
            nc.vector.memset(ones[:], 1.0)
            banks = [psum.tile([C, 512], f32, name=f"bank{b}",
                               tag=f"bank{b}") for b in range(NB)]

            for ti, (b, tk) in enumerate(_tiles(M)):
                x = xpool.tile([128, tk, C], f32, tag="x")
                nc.sync.dma_start(
                    x[:],
                    yp.ap()[b * 128 : (b + tk) * 128, :].rearrange(
                        "(p k) c -> p k c", k=tk
                    ),
                )
                rmax = spool.tile([128, tk], f32, tag="rmax")
                nc.vector.tensor_reduce(
                    rmax[:], x[:], axis=mybir.AxisListType.X, op=Alu.max
                )
                anti = apool.tile([128, tk, C], bf16, tag="anti")
                j = min(J_DVE, tk)
                nc.vector.tensor_tensor(
                    anti[:, 0:j, :], x[:, 0:j, :],
                    rmax[:, 0:j, None].broadcast_to([128, j, C]),
                    op=Alu.is_lt,
                )
                g = _n_gs(ti, tk, j)
                for k in range(j, j + g):
                    nc.gpsimd.tensor_scalar(
                        anti[:, k, :], x[:, k, :], rmax[:, k : k + 1], None,
                        op0=Alu.is_lt,
                    )
                for k in range(j + g, tk):
                    nc.scalar.activation(
                        anti[:, k, :], x[:, k, :], Act.Sign,
                        bias=rmax[:, k : k + 1], scale=-1.0,
                    )
                for k in range(tk):
                    G = b + k
                    nc.tensor.matmul(
                        banks[G // 512][:, (G % 512) : (G % 512) + 1],
                        anti[:, k, :], ones[:],
                        start=True, stop=True,
                    )

            for bi in range(NB):
                w = min(512, M - bi * 512)
                sb = spool.tile([C, 512], f32, tag=f"osb{bi}")
                nc.scalar.copy(sb[:, 0:w], banks[bi][:, 0:w])
                nc.sync.dma_start(out.ap()[bi, :, 0:w], sb[:, 0:w])

    nc.compile()
    return nc


def _get_nc(M):
    if M not in _CACHE:
        _CACHE[M] = _build(M)
    return _CACHE[M]


def _layout(y_true):
    """Class-sorted chunk layout. Returns (src, chunk_class, n_c, chunks_c, M).

    src[g*128 + p] = original row index of row p of logical chunk g
    (-1 for sentinel rows).  chunk_class[g] in [0,C) or C for all-sentinel
    dummy chunks.
    """
    yt = np.asarray(y_true).astype(np.int64).ravel()
    n_c = np.bincount(yt, minlength=C).astype(np.int64)
    chunks_c = (n_c + 127) // 128
    M_total = int(chunks_c.sum())
    M = -(-M_total // NCORES)
    total_chunks = M * NCORES

    order = np.argsort(yt, kind="stable").astype(np.int64)
    starts = np.zeros(C, np.int64)
    starts[1:] = np.cumsum(chunks_c)[:-1]
    src = np.full(total_chunks * 128, -1, np.int64)
    dst = np.concatenate(
        [starts[c] * 128 + np.arange(n_c[c]) for c in range(C)]
    )
    src[dst] = order

    chunk_class = np.full(total_chunks, C, np.int64)
    chunk_class[:M_total] = np.repeat(np.arange(C), chunks_c)
    return src, chunk_class, n_c, chunks_c, M


def _shards(y_pred, src, M):
    """Per-core physical shards in the device's [p, k] tile layout."""
    yp = np.ascontiguousarray(np.asarray(y_pred, dtype=np.float32))
    tiles = _tiles(M)
    shards = []
    for i in range(NCORES):
        sc = src[i * M * 128 : (i + 1) * M * 128]
        phys = np.empty(M * 128, np.int64)
        for (b, tk) in tiles:
            blk = sc[b * 128 : (b + tk) * 128].reshape(tk, 128)
            phys[b * 128 : (b + tk) * 128] = blk.T.ravel()
        mask = phys < 0
        shard = yp[np.where(mask, 0, phys)]
        if mask.any():
            shard[mask] = 0.0
            shard[mask, 0] = BIG
        shards.append(np.ascontiguousarray(shard))
    return shards


def _run(y_pred, y_true, trace=False):
    src, chunk_class, n_c, chunks_c, M = _layout(y_true)
    nc = _get_nc(M)
    shards = _shards(y_pred, src, M)
    in_maps = [{"yp": s} for s in shards]
    res = None
    for attempt in range(3):
        try:
            res = bass_utils.run_bass_kernel_spmd(
                nc, in_maps, core_ids=list(range(NCORES)), trace=trace
            )
            break
        except Exception:
            if attempt == 2:
                raise
            time.sleep(2.0)

    NB = -(-M // 512)
    counts_all = []
    for r in res.results:
        o = r["out"].astype(np.float64)            # [NB, C, 512]
        cs = o.transpose(0, 2, 1).reshape(NB * 512, C)[:M]  # colsums [M, C]
        counts_all.append(128.0 - cs)
    counts_all = np.concatenate(counts_all, 0)     # [8M, C]
    cm = np.zeros((C + 1, C), np.float64)
    np.add.at(cm, chunk_class, counts_all)
    cm = cm[:C]
    cm[:, 0] -= (chunks_c * 128 - n_c)             # sentinel rows -> pred 0
    diag = np.diagonal(cm)
    precision = diag / (cm.sum(axis=1) + EPS)
    recall = diag / (cm.sum(axis=0) + EPS)
    f1 = 2.0 * precision * recall / (precision + recall + EPS)
    return np.float32(f1.mean()), res


def kernel(y_pred, y_true):
    out, _ = _run(y_pred, y_true, trace=False)
    return out
